# revision 13
# baseline (speedup 1.0000x reference)
"""Trainium2 Bass kernel for nn_AttributeMultiHeadedAttention.

Reference math (B=4, L=1024, A=8, D=1024, H=16, DK=64):
  q = proj(query, Wq, bq); k = proj(key, Wk, bk); v = proj(value, Wv, bv)
  per (b, l, h): softmax over the A=8 attribute axis, head dim DK=64
  out = (attn @ v) reassembled, then @ Wo.T + bo

Strategy (576 us baseline -> ~565 us; vs 758 us previous session):
  - Shard the B*L = 4096 (b,l) groups across 8 cores (512 groups = 4096
    rows of [1024] per core). Groups are independent: no collectives.
  - Host prep: inputs transposed to feature-major [D, rows], bf16;
    weights pre-transposed to [d, e]; bv/bo folded on host (softmax
    rows sum to 1); Wo rows permuted to the device's batch-order head
    layout. bk is dropped entirely: it shifts every score row by
    q_i.bk, constant along the softmax axis -> exactly invariant.
  - Device per core, per 512-row super-chunk: Q^T/K^T projections with
    ACT evictions (bias via per-partition ACT bias); V row-major with
    a ones column per head (ACT copy). All PSUM evictions stay on ACT:
    putting them on DVE slows the PE matmul stream ~25% (SBUF port
    contention). Attention per 128-row chunk: 4 same-parity heads'
    grams batched into one [128,512] PSUM bank (consecutive matmuls
    into one bank MUST share the operand partition offset or the HW
    errors out) -> ONE 512-wide exp (ACT, (FD+352)/1.2ns makes wide
    activations cheap) + ONE mask multiply (DVE) per batch; per head
    PV = E_h.T @ [V_h|1] -> unnormalized O + denominator; batched
    normalization: one reciprocal [P,4,1] + one broadcast tensor_mul
    per 4-head batch; PE-transpose O to O^T; Y = O^T.T @ Wo^T (ACT
    evict, bf16 out halves the store traffic).
  - Pipeline: attention for super-chunk s is issued after the
    projections of s+1; within attention, chunk c's grams are issued
    before chunk c-1's PV/transpose stage and chunk c-2's out-proj
    (the extra phase of slack hides the exp->mask latency and the
    ysb-eviction WAR on the shared ps512 pool). A dependency-free
    matmul burst bridges the DMA prologue so the PE_HAM clock gate is
    warm (2.4 GHz) when the real stream starts; DMA issue order is
    tuned (xq0 + Wq halves first, Wo last) because the Sync queue
    serializes DMA issue at ~0.6-2.8 us each.
"""

import numpy as np
import ml_dtypes

B, L, A, D, H = 4, 1024, 8, 1024, 16
DK = D // H
NCORES = 8
GROUPS_PER_CORE = B * L // NCORES      # 512
R = GROUPS_PER_CORE * A                # 4096 rows per core
P = 128
SC = 512                               # super-chunk rows
BF16 = ml_dtypes.bfloat16
F8 = ml_dtypes.float8_e4m3             # TRN fp8_e4m3 (max 240)

# fp8 scales for the Q/K projections (powers of 2; folded into bq and the
# exp scale).  x*32 keeps randn in e4m3 normal range (max|x|*32 ~ 173 <
# 240); W*4096 lifts U(+-1/32) out of the subnormals (max 128 < 240).
SX = 32.0
SW = 4096.0
QK_FP8 = True

_CACHE = {}

# build-config flags (bisect aids)
V_EVICT_ACT = True     # V eviction on ACT (else DVE tensor_copy)
Y_BF16 = True          # bf16 output DMA (else f32)
EXP_BATCH = True       # 4-head gram batches + one exp/mask per batch
QK_EVICT_DVE = True    # q/k evictions on DVE (else ACT)
EXP_WIDE = True        # one 512-wide exp per batch (else 4 narrow exps)
GRAM_ONE_GROUP = True  # 4 grams as one accumulation group (else 4 groups)

NORM_BATCH = True      # batched reciprocal + broadcast-mul normalization

# heads grouped by parity: consecutive matmuls into one PSUM bank must use
# the same operand partition offset (alternating offsets hangs the PE)
HEAD_BATCHES = ((0, 2, 4, 6), (1, 3, 5, 7), (8, 10, 12, 14), (9, 11, 13, 15))
# oc/ot hold heads in batch order; Wo rows are permuted on host to match
HEAD_PERM = [h for hb in HEAD_BATCHES for h in hb]


def _build(n_rows=R):
    import concourse.mybir as mybir
    import concourse.tile as tile
    from concourse import bacc

    f32 = mybir.dt.float32
    bf16 = mybir.dt.bfloat16
    fp8 = mybir.dt.float8e4
    AF = mybir.ActivationFunctionType
    DR = mybir.MatmulPerfMode.DoubleRow

    nc = bacc.Bacc("TRN2", target_bir_lowering=False, debug=False)

    n_sc = n_rows // SC
    # exp scale: 1/sqrt(DK) plus undo of the fp8 operand scales (q and k
    # are both scaled by SX*SW = 2^17 -> scores carry 2^34)
    exp_scale = 0.125 / (SX * SW) ** 2 if QK_FP8 else 0.125

    # xq/xk arrive pre-tiled host-side as [s, p, k, two, c] fp8 so each
    # super-chunk DMA is one contiguous block with 4 KB partition lines;
    # d = k*256 + two*128 + p (DoubleRow packs two 128-deep k-subtiles).
    xq = nc.dram_tensor("xq_t", (n_sc, P, 4, 2, SC), fp8,
                        kind="ExternalInput").ap()
    xk = nc.dram_tensor("xk_t", (n_sc, P, 4, 2, SC), fp8,
                        kind="ExternalInput").ap()
    xv = nc.dram_tensor("xv_t", (D, n_rows), bf16, kind="ExternalInput").ap()
    wq = nc.dram_tensor("wq_t", (P, 4, 2, D), fp8, kind="ExternalInput").ap()
    wk = nc.dram_tensor("wk_t", (P, 4, 2, D), fp8, kind="ExternalInput").ap()
    wv = nc.dram_tensor("wv_t", (D, D), bf16, kind="ExternalInput").ap()
    wo = nc.dram_tensor("wo_t", (D, D), bf16, kind="ExternalInput").ap()
    bq = nc.dram_tensor("bq2", (P, 8), f32, kind="ExternalInput").ap()
    msk = nc.dram_tensor("msk4", (P, 512), bf16, kind="ExternalInput").ap()
    idn = nc.dram_tensor("idn", (P, P), bf16, kind="ExternalInput").ap()
    y_dt = bf16 if Y_BF16 else f32
    y = nc.dram_tensor("y", (n_rows, D), y_dt, kind="ExternalOutput").ap()

    with tile.TileContext(nc) as tc:
        with tc.tile_pool(name="const", bufs=1) as constp, \
             tc.tile_pool(name="xin", bufs=2) as xinp, \
             tc.tile_pool(name="qkt", bufs=2) as qktp, \
             tc.tile_pool(name="vc", bufs=8) as vcp, \
             tc.tile_pool(name="oc", bufs=3) as ocp, \
             tc.tile_pool(name="ot", bufs=3) as otp, \
             tc.tile_pool(name="ep", bufs=8) as ep, \
             tc.tile_pool(name="rp", bufs=16) as rp, \
             tc.tile_pool(name="yp", bufs=2) as yp, \
             tc.tile_pool(name="ps512", bufs=2, space="PSUM") as ps512, \
             tc.tile_pool(name="psg", bufs=2, space="PSUM") as psgp, \
             tc.tile_pool(name="pso", bufs=2, space="PSUM") as psop, \
             tc.tile_pool(name="pst", bufs=2, space="PSUM") as pstp:

            xv_r = xv.rearrange("(k p) n -> p k n", p=P)

            def dma_input(x_r, tag):
                xs = xinp.tile([P, 8, SC], bf16, tag=tag, name=tag)
                nc.sync.dma_start(xs[:], x_r)
                return xs

            def dma_input8(x5, s, tag):
                xs = xinp.tile([P, 4, 2, SC], fp8, tag=tag, name=tag)
                nc.sync.dma_start(xs[:], x5[s])
                return xs

            def dma_inputs(s):
                ssl = slice(s * SC, (s + 1) * SC)
                return (dma_input8(xq, s, "xq"),
                        dma_input8(xk, s, "xk"),
                        dma_input(xv_r[:, :, ssl], "xv"))

            # DMA issue order tuned so projection chain j only waits for
            # xq(s0) + its own e-chunk of Wq (~1.25 MB), not the full
            # 11 MB of weights: per-j weight tiles, Wo last.
            # Dependency-free matmul burst during the DMA prologue keeps
            # the PE_HAM activity window busy so the real stream starts
            # at 2.4 GHz instead of the cold 1.2 GHz throttle.
            warm_sb = constp.tile([P, P], bf16, tag="warm")
            nc.vector.memset(warm_sb[:], 0.0)
            warm_ps = pstp.tile([P, P], f32, tag="pst")
            # ~4us of burst: enough for the PE_HAM full-speed ramp (3us)
            # and ends about when the xq0+wq DMAs land (~13.5us) instead
            # of stalling the real stream until 20us like the old 140.
            for _ in range(56):
                nc.tensor.matmul(warm_ps[:], warm_sb[:], warm_sb[:],
                                 start=True, stop=True)

            # The Sync queue serializes DMA *issue* (~0.6-2.8us each), so
            # front-load only what the first chains need: xq0, then the
            # first Wq half; everything else streams behind the PE.
            s0 = slice(0, SC)
            xq0 = dma_input8(xq, 0, "xq")
            wq_tiles = []
            for piece, (e0, e1) in enumerate(((0, 512), (512, 1024))):
                w_s = constp.tile([P, 4, 2, e1 - e0], fp8, tag=f"wq{piece}",
                                  name=f"wq{piece}")
                nc.sync.dma_start(w_s[:], wq[:, :, :, e0:e1])
                wq_tiles.append((e0, w_s))

            def wq_slice(k, j):
                e = j * P
                for e0, w_s in reversed(wq_tiles):
                    if e >= e0:
                        return w_s[:, k, :, e - e0:e - e0 + P]
                raise AssertionError
            xk0 = dma_input8(xk, 0, "xk")
            bq_s = constp.tile([P, 8], f32, tag="bq")
            nc.sync.dma_start(bq_s[:], bq)
            wk_s = constp.tile([P, 4, 2, D], fp8, tag="wk")
            nc.sync.dma_start(wk_s[:], wk)
            xv0 = dma_input(xv_r[:, :, s0], "xv")
            wv_s = constp.tile([P, 8, D], bf16, tag="wv")
            nc.sync.dma_start(wv_s[:], wv.rearrange("(k p) e -> p k e", p=P))
            msk_s = constp.tile([P, 512], bf16, tag="msk")
            nc.sync.dma_start(msk_s[:], msk)
            idn_s = constp.tile([P, P], bf16, tag="idn")
            nc.sync.dma_start(idn_s[:], idn)
            wo_s = constp.tile([P, 8, D], bf16, tag="wo")
            nc.sync.dma_start(
                wo_s[:], wo.rearrange("(k p) e -> p k e", p=P)
            )

            pending_out = []

            def attention_sc(qt, kt, vts, s, chains):
                while pending_out:
                    pending_out.pop(0)()
                # Per 128-row chunk c: grams of all 16 heads first (in 4
                # batches of 4 heads sharing one PSUM bank -> one exp +
                # one mask op each); the PV/normalize/transpose/out-proj
                # block for chunk c-1 is issued after chunk c's grams.
                ets = {}

                def gram_chunk(c):
                    csl = slice(c * P, (c + 1) * P)
                    if EXP_BATCH and GRAM_ONE_GROUP:
                        # Pairs of head batches (even-parity po=0, odd po=64)
                        # interleaved into two banks: the disjoint row-groups
                        # execute concurrently on the split PE array (~2x).
                        # Same-bank consecutive matmuls still share their
                        # operand partition offset (HW constraint).
                        for pair in range(2):
                            hbA, hbB = 2 * pair, 2 * pair + 1
                            psgA = psgp.tile([P, 512], f32, tag="psg")
                            psgB = psgp.tile([P, 512], f32, tag="psg")
                            for i in range(4):
                                for hb, psg in ((hbA, psgA), (hbB, psgB)):
                                    h = HEAD_BATCHES[hb][i]
                                    j, po = h // 2, (h % 2) * DK
                                    nc.tensor.matmul(
                                        psg[:, i * P:(i + 1) * P],
                                        kt[po:po + DK, j, csl],
                                        qt[po:po + DK, j, csl],
                                        start=(i == 0),
                                        stop=(i == 3),
                                        skip_group_check=True,
                                    )
                            for hb, psg in ((hbA, psgA), (hbB, psgB)):
                                et = ep.tile([P, 512], bf16, tag="et")
                                eu = ep.tile([P, 512], bf16, tag="eu")
                                nc.scalar.activation(
                                    eu[:], psg[:], AF.Exp, scale=exp_scale
                                )
                                nc.vector.tensor_mul(et[:], eu[:], msk_s[:])
                                ets[(c, hb)] = et
                        return
                    for hb in range(4):
                        heads = HEAD_BATCHES[hb]
                        if EXP_BATCH:
                            # 4 heads of equal parity per bank: consecutive
                            # matmuls into one PSUM bank must share the
                            # operand partition offset (HW constraint)
                            psg = psgp.tile([P, 512], f32, tag="psg")
                            for i, h in enumerate(heads):
                                j, po = h // 2, (h % 2) * DK
                                nc.tensor.matmul(
                                    psg[:, i * P:(i + 1) * P],
                                    kt[po:po + DK, j, csl],
                                    qt[po:po + DK, j, csl],
                                    start=(i == 0) if GRAM_ONE_GROUP else True,
                                    stop=(i == 3) if GRAM_ONE_GROUP else True,
                                    skip_group_check=GRAM_ONE_GROUP,
                                )
                            et = ep.tile([P, 512], bf16, tag="et")
                            if EXP_WIDE:
                                eu = ep.tile([P, 512], bf16, tag="eu")
                                nc.scalar.activation(
                                    eu[:], psg[:], AF.Exp, scale=exp_scale
                                )
                                nc.vector.tensor_mul(et[:], eu[:], msk_s[:])
                            else:
                                for i in range(4):
                                    isl = slice(i * P, (i + 1) * P)
                                    eu = ep.tile([P, P], bf16, tag="eu")
                                    nc.scalar.activation(
                                        eu[:], psg[:, isl], AF.Exp,
                                        scale=exp_scale,
                                    )
                                    nc.vector.tensor_mul(
                                        et[:, isl], eu[:], msk_s[:, 0:P]
                                    )
                        else:
                            et = ep.tile([P, 512], bf16, tag="et")
                            for i, h in enumerate(heads):
                                j, po = h // 2, (h % 2) * DK
                                psg = psgp.tile([P, P], f32, tag="psg")
                                nc.tensor.matmul(
                                    psg[:],
                                    kt[po:po + DK, j, csl],
                                    qt[po:po + DK, j, csl],
                                    start=True,
                                    stop=True,
                                )
                                eu = ep.tile([P, P], bf16, tag="eu")
                                nc.scalar.activation(
                                    eu[:], psg[:], AF.Exp, scale=exp_scale
                                )
                                nc.vector.tensor_mul(
                                    et[:, i * P:(i + 1) * P], eu[:],
                                    msk_s[:, 0:P],
                                )
                        ets[(c, hb)] = et

                ots = {}

                def pvt_chunk(c):
                    vt = vts[c]
                    oc = ocp.tile([P, 16, DK], bf16, tag="oc")
                    for hb in range(4):
                        heads = HEAD_BATCHES[hb]
                        et = ets.pop((c, hb))
                        pso = psop.tile([P, 4, 65], f32, tag="pso")
                        for i, h in enumerate(heads):
                            nc.tensor.matmul(
                                pso[:, i, :], et[:, i * P:(i + 1) * P],
                                vt[:, h, :],
                                start=True, stop=True,
                            )
                        if NORM_BATCH:
                            # oc slots are in batch order (host permutes Wo)
                            rt = rp.tile([P, 4, 1], f32, tag="r")
                            nc.vector.reciprocal(rt[:], pso[:, :, 64:65])
                            nc.vector.tensor_mul(
                                oc[:, 4 * hb:4 * hb + 4, :],
                                pso[:, :, 0:DK],
                                rt[:].broadcast_to([P, 4, DK]),
                            )
                        else:
                            for i, h in enumerate(heads):
                                rt = rp.tile([P, 1], f32, tag="r")
                                nc.vector.reciprocal(rt[:], pso[:, i, 64:65])
                                nc.vector.tensor_scalar_mul(
                                    oc[:, 4 * hb + i, :], pso[:, i, 0:DK],
                                    rt[:]
                                )

                    ot = otp.tile([P, 8, P], bf16, tag="ot")
                    ocf = oc[:].rearrange("p h d -> p (h d)")
                    for t in range(8):
                        pst = pstp.tile([P, P], bf16, tag="pst")
                        nc.tensor.transpose(
                            pst[:], ocf[:, t * P:(t + 1) * P], idn_s[:]
                        )
                        nc.vector.tensor_copy(ot[:, t, :], pst[:])
                    ots[c] = ot

                def out_chunk(c):
                    out_chunk_impl(c)

                def out_chunk_fn(c):
                    ot = ots.pop(c)
                    return lambda fine=False: out_chunk_impl(c, ot, fine)

                def out_chunk_impl(c, ot=None, fine=False):
                    # out-proj deferred one more chunk: its ps512 tiles are
                    # evicted a full phase before reuse (no WAR stall);
                    # per-half DMA so eh0's store overlaps eh1's chain.
                    # fine=True (kernel tail only): quarter-width evict+DMA
                    # so the final stores drain while the PE still runs.
                    if ot is None:
                        ot = ots.pop(c)
                    ysb = yp.tile([P, D], bf16 if Y_BF16 else f32, tag="y")
                    row0 = s * SC + c * P
                    for eh in range(2):
                        esl = slice(eh * 512, (eh + 1) * 512)
                        ps = ps512.tile([P, SC], f32, tag="ps512")
                        for k in range(8):
                            nc.tensor.matmul(
                                ps[:],
                                ot[:, k, :],
                                wo_s[:, k, esl],
                                start=(k == 0),
                                stop=(k == 7),
                            )
                        if fine:
                            for sub in range(2):
                                qsl = slice(eh * 512 + sub * 256,
                                            eh * 512 + (sub + 1) * 256)
                                nc.scalar.activation(
                                    ysb[:, qsl], ps[:, sub * 256:(sub + 1) * 256],
                                    AF.Copy,
                                )
                                nc.sync.dma_start(
                                    y[row0:row0 + P, qsl], ysb[:, qsl]
                                )
                        else:
                            nc.scalar.activation(ysb[:, esl], ps[:], AF.Copy)
                            nc.sync.dma_start(
                                y[row0:row0 + P, esl], ysb[:, esl]
                            )

                for c in range(4):
                    # weave 6 of the next SC's projection chains ahead of
                    # each attention chunk: spreads ACT/DVE attention load
                    # across the whole SC instead of a saturated burst
                    for t in chains[6 * c:6 * (c + 1)]:
                        t()
                    gram_chunk(c)
                    if c > 0:
                        pvt_chunk(c - 1)
                    if c > 1:
                        out_chunk(c - 2)
                for t in chains[24:]:
                    t()
                pvt_chunk(3)
                out_chunk(2)
                # last out-chunk runs after the next SC's projections: its
                # ps512 slot then recycles through the proj ring (no
                # back-to-back WAR with out_chunk(2)'s eviction)
                pending_out.append(out_chunk_fn(3))

            def q_chain(xq_s, qt, j):
                def run():
                    ps = ps512.tile([P, SC], f32, tag="ps512")
                    for k in range(4):
                        nc.tensor.matmul(
                            ps[:],
                            wq_slice(k, j),
                            xq_s[:, k, :, :],
                            start=(k == 0),
                            stop=(k == 3),
                            perf_mode=DR,
                        )
                    if QK_EVICT_DVE:
                        nc.vector.tensor_scalar_add(
                            qt[:, j, :], ps[:], bq_s[:, j:j + 1]
                        )
                    else:
                        nc.scalar.activation(
                            qt[:, j, :], ps[:], AF.Identity,
                            bias=bq_s[:, j:j + 1],
                        )
                return run

            def k_chain(xk_s, kt, j):
                def run():
                    ps = ps512.tile([P, SC], f32, tag="ps512")
                    for k in range(4):
                        nc.tensor.matmul(
                            ps[:],
                            wk_s[:, k, :, j * P:(j + 1) * P],
                            xk_s[:, k, :, :],
                            start=(k == 0),
                            stop=(k == 3),
                            perf_mode=DR,
                        )
                    if QK_EVICT_DVE:
                        nc.vector.tensor_copy(kt[:, j, :], ps[:])
                    else:
                        nc.scalar.activation(kt[:, j, :], ps[:], AF.Copy)
                return run

            def v_chain(xv_s, vt, rb, eh):
                def run():
                    if eh == 0:
                        nc.vector.memset(vt[:, :, 64:65], 1.0)
                    ps = ps512.tile([P, SC], f32, tag="ps512")
                    for k in range(8):
                        nc.tensor.matmul(
                            ps[:],
                            xv_s[:, k, rb * P:(rb + 1) * P],
                            wv_s[:, k, eh * 512:(eh + 1) * 512],
                            start=(k == 0),
                            stop=(k == 7),
                        )
                    if V_EVICT_ACT:
                        nc.scalar.activation(
                            vt[:, eh * 8:(eh + 1) * 8, 0:64],
                            ps[:].rearrange("p (h d) -> p h d", h=8),
                            AF.Copy,
                        )
                    else:
                        nc.vector.tensor_copy(
                            vt[:, eh * 8:(eh + 1) * 8, 0:64],
                            ps[:].rearrange("p (h d) -> p h d", h=8),
                        )
                return run

            def build_chains(s, xq_s, xk_s, xv_s, qt, kt, vts):
                chains = [q_chain(xq_s, qt, j) for j in range(8)]
                if s + 1 < n_sc:
                    # prefetch issued after the Q chains: keeps the
                    # prologue's critical weight DMAs uncontended
                    def prefetch():
                        nxt_holder[0] = dma_inputs(s + 1)
                    chains.append(prefetch)
                chains += [k_chain(xk_s, kt, j) for j in range(8)]
                for rb in range(4):
                    vt = vcp.tile([P, 16, 65], bf16, tag="vc")
                    vts.append(vt)
                    chains += [v_chain(xv_s, vt, rb, eh) for eh in range(2)]
                return chains

            prev = None
            nxt_holder = [(xq0, xk0, xv0)]
            for s in range(n_sc):
                xq_s, xk_s, xv_s = nxt_holder[0]
                qt = qktp.tile([P, 8, SC], bf16, tag="qt")
                kt = qktp.tile([P, 8, SC], bf16, tag="kt")
                vts = []
                chains = build_chains(s, xq_s, xk_s, xv_s, qt, kt, vts)
                if prev is None:
                    for t in chains:
                        t()
                else:
                    # attention for s-1 woven with the projections of s
                    attention_sc(*prev, chains)
                prev = (qt, kt, vts, s)

            attention_sc(*prev, [])
            while pending_out:
                pending_out.pop(0)(True)

    nc.compile()
    return nc


def _fp8_x(x2, n_rows):
    """[rows, D] f32 -> fp8 [n_sc, P, 4, 2, SC]; d = k*256 + two*128 + p."""
    n_sc = n_rows // SC
    x8 = (x2 * SX).astype(F8)
    x8 = x8.reshape(n_sc, SC, 4, 2, P).transpose(0, 4, 2, 3, 1)
    return np.ascontiguousarray(x8)


def _fp8_w(W):
    """W [e, d] -> fp8 W.T tiled [P, 4, 2, D]; d = k*256 + two*128 + p."""
    w8 = (np.asarray(W, np.float32).T * SW).astype(F8)        # [d, e]
    w8 = w8.reshape(4, 2, P, D).transpose(2, 0, 1, 3)
    return np.ascontiguousarray(w8)


def _host_inputs(query, key, value, Wq, bq, Wk, Wv, Wo, n_rows=R):
    """Per-core in_maps. query/key/value: [B, L, A, D] float32."""
    xs = {}
    for name, x in (("xq_t", query), ("xk_t", key), ("xv_t", value)):
        xs[name] = np.asarray(x, np.float32).reshape(-1, D)
    n_cores = xs["xq_t"].shape[0] // n_rows
    # oc/ot tiles hold heads in batch order; permute Wo's d-rows to match
    wo_t = np.asarray(Wo, np.float32).T.reshape(H, DK, D)[HEAD_PERM]
    wo_t = wo_t.reshape(D, D)
    shared = {
        "wq_t": _fp8_w(Wq),
        "wk_t": _fp8_w(Wk),
        "wv_t": np.ascontiguousarray(np.asarray(Wv, np.float32).T).astype(BF16),
        "wo_t": np.ascontiguousarray(wo_t).astype(BF16),
        "bq2": np.ascontiguousarray(
            np.asarray(bq, np.float32).reshape(8, P).T) * (SX * SW),
        "msk4": np.tile(
            np.kron(np.eye(16, dtype=np.float32), np.ones((8, 8), np.float32)),
            (1, 4),
        ).astype(BF16),
        "idn": np.eye(P, dtype=np.float32).astype(BF16),
    }
    in_maps = []
    for c in range(n_cores):
        m = dict(shared)
        for name in ("xq_t", "xk_t"):
            shard = xs[name][c * n_rows:(c + 1) * n_rows]  # [n_rows, D] f32
            m[name] = _fp8_x(shard, n_rows)
        shard = xs["xv_t"][c * n_rows:(c + 1) * n_rows].astype(BF16)
        m["xv_t"] = np.ascontiguousarray(shard.T)          # [D, n_rows]
        in_maps.append(m)
    return in_maps


def kernel(query, key, value, Wq, bq, Wk, bk, Wv, bv, Wo, bo, d_atrbt):
    assert int(d_atrbt) == A
    from concourse.bass_utils import run_bass_kernel_spmd

    if "nc" not in _CACHE:
        _CACHE["nc"] = _build(R)
    nc = _CACHE["nc"]

    in_maps = _host_inputs(query, key, value, Wq, bq, Wk, Wv, Wo)
    res = run_bass_kernel_spmd(nc, in_maps, core_ids=list(range(NCORES)))
    _CACHE["last_results"] = res

    Wo_f = np.asarray(Wo, np.float32)
    host_bias = Wo_f @ np.asarray(bv, np.float32) + np.asarray(bo, np.float32)
    parts = [res.results[c]["y"] for c in range(NCORES)]
    out = np.concatenate(parts, axis=0).astype(np.float32)  # [B*L*A, D]
    out = out + host_bias[None, :]
    return out.reshape(B, L, A, D).astype(np.float32)

